# revision 26
# baseline (speedup 1.0000x reference)
"""Trainium2 Bass kernel for nn_DistanceLoss.

Computes: sum over batch of ||centers[argmax(pred, -1)] - centers[true]|| / 255

Strategy (data-parallel over 8 NeuronCores, B=65536 rows split 8192/core):
  - Stream pred shard through SBUF in 64 tiles of [128 rows, 1000 classes]
    on the SP (sync) HWDGE queue, 8-slot ring buffer.
  - Argmax per row with exactly ONE vector-engine pass over the data:
      * DVE: running-max scan (tensor_tensor_scan, op0=max, op1=bypass).
      * ACT: idx = sum_t sign(rowmax - cummax[t]) (counts elements strictly
        before the first position attaining the max == jnp.argmax index,
        first-index tie-break included) via one activation with accum_out
        on the otherwise-idle scalar engine.
  - Pred-side center lookup: per-tile [P,1] indirect DMA gathers on gpsimd
    (the only gather available in the standard ucode library), pipelined in
    groups of 8 behind the scalar-engine index production so they hide
    under the scan stream. True-side lookup is input-only, so it is
    precomputed on the host and DMA'd in as a [128, 64, 2] input.
  - Tiny fused distance epilogue sqrt((dx^2+dy^2)/255^2) with row-sum
    accumulation; each core emits [128] partial sums; host reduces 8x128.

Raw bass blocks with explicit semaphores (no TileContext): walrus's
direct2d pseudo-DMA encodes at most one attached sync-wait, so waits are
issued as separate engine instructions instead.
"""

import sys
from contextlib import ExitStack

import numpy as np

if "/opt/trn_rl_repo" not in sys.path:  # harness-proof import of concourse
    sys.path.insert(0, "/opt/trn_rl_repo")

B = 65536
C = 1000
N_CORES = 8
ROWS_PER_CORE = B // N_CORES          # 8192
P = 128                               # SBUF partitions
T = ROWS_PER_CORE // P                # 64 tiles per core
SLOTS = 8                             # pred ring slots
CMX = 4                               # cummax ring slots
GRP = 8                               # gather group size (tiles)

_CACHE = {}


def _build():
    import concourse.bass as bass
    from concourse import mybir

    FP32 = mybir.dt.float32
    U32 = mybir.dt.uint32
    Act = mybir.ActivationFunctionType
    Alu = mybir.AluOpType

    nc = bass.Bass()
    pred_d = nc.declare_dram_parameter("pred", [ROWS_PER_CORE, C], FP32, isOutput=False)
    cb_d = nc.declare_dram_parameter("cb_pre", [P, T, 2], FP32, isOutput=False)
    cent_d = nc.declare_dram_parameter("centers", [C, 2], FP32, isOutput=False)
    out_d = nc.declare_dram_parameter("partial", [P, 1], FP32, isOutput=True)

    with ExitStack() as ctx:
        x_buf = ctx.enter_context(nc.sbuf_tensor("x_buf", [P, SLOTS * C], FP32))
        m8_buf = ctx.enter_context(nc.sbuf_tensor("m8_buf", [P, 16], FP32))
        idx8 = ctx.enter_context(nc.sbuf_tensor("idx8", [P, T * 8], U32))
        ca = ctx.enter_context(nc.sbuf_tensor("ca", [P, T, 2], FP32))
        cb = ctx.enter_context(nc.sbuf_tensor("cb", [P, T, 2], FP32))
        d2 = ctx.enter_context(nc.sbuf_tensor("d2", [P, T, 2], FP32))
        s2 = ctx.enter_context(nc.sbuf_tensor("s2", [P, T], FP32))
        dist = ctx.enter_context(nc.sbuf_tensor("dist", [P, T], FP32))
        part_sb = ctx.enter_context(nc.sbuf_tensor("part_sb", [P, 1], FP32))

        block = ctx.enter_context(nc.Block())
        s_x = [ctx.enter_context(nc.semaphore(f"s_x{i}")) for i in range(SLOTS)]
        s_scan = ctx.enter_context(nc.semaphore("s_scan"))
        s_m8 = ctx.enter_context(nc.semaphore("s_m8"))
        s_cb = ctx.enter_context(nc.semaphore("s_cb"))
        s_g = ctx.enter_context(nc.semaphore("s_g"))
        s_eps = ctx.enter_context(nc.semaphore("s_eps"))
        s_fin = ctx.enter_context(nc.semaphore("s_fin"))
        s_out = ctx.enter_context(nc.semaphore("s_out"))

        def xs(t):
            return x_buf[:, (t % SLOTS) * C:(t % SLOTS) * C + C]

        def m8(t):
            return m8_buf[:, (t % 2) * 8:(t % 2) * 8 + 8]

        @block.sync
        def _(sp):
            sp.dma_start(out=cb[:], in_=cb_d[:]).then_inc(s_cb, 16)
            for t in range(T):
                if t >= SLOTS:
                    # slot free once its previous tile's scan completed
                    sp.wait_ge(s_scan, t - SLOTS + 1)
                sp.dma_start(out=xs(t), in_=pred_d[t * P:(t + 1) * P, :]).then_inc(
                    s_x[t % SLOTS], 16
                )
            sp.wait_ge(s_fin, 1)
            sp.dma_start(out=out_d[:], in_=part_sb[:]).then_inc(s_out, 16)
            sp.wait_ge(s_out, 16)

        @block.vector
        def _(v):
            for t in range(T):
                v.wait_ge(s_x[t % SLOTS], 16 * (t // SLOTS + 1))
                if t >= 2:
                    # m8 slot WAW/WAR sync (write buffers drain out of order)
                    v.wait_ge(s_scan, t - 1)
                v.max(m8(t), xs(t)).then_inc(s_m8, 1)
                v.wait_ge(s_m8, t + 1)
                v.max_index(
                    idx8[:, 8 * t:8 * t + 8], m8(t), xs(t)
                ).then_inc(s_scan, 1)
            v.wait_ge(s_g, 16 * T)
            v.wait_ge(s_cb, 16)
            v.tensor_tensor(out=d2[:], in0=ca[:], in1=cb[:], op=Alu.subtract).then_inc(
                s_eps, 1
            )
            v.wait_ge(s_eps, 1)
            v.tensor_tensor(out=d2[:], in0=d2[:], in1=d2[:], op=Alu.mult).then_inc(
                s_eps, 1
            )
            v.wait_ge(s_eps, 2)
            v.tensor_tensor(
                out=s2[:], in0=d2[:, :, 0], in1=d2[:, :, 1], op=Alu.add
            ).then_inc(s_eps, 1)

        @block.scalar
        def _(act):
            act.wait_ge(s_eps, 3)
            act.activation(
                out=dist[:],
                in_=s2[:],
                func=Act.Sqrt,
                scale=1.0 / (255.0 * 255.0),
                accum_out=part_sb[:],
            ).then_inc(s_fin, 1)

        @block.gpsimd
        def _(g):
            # Per-tile [P,1] gathers pipelined directly behind max_index
            for t in range(T):
                g.wait_ge(s_scan, t + 1)
                g.indirect_dma_start(
                    out=ca[:, t, :],
                    out_offset=None,
                    in_=cent_d[:],
                    in_offset=bass.IndirectOffsetOnAxis(
                        ap=idx8[:, 8 * t:8 * t + 1], axis=0
                    ),
                ).then_inc(s_g, 16)

    return nc


def _get_nc():
    if "nc" not in _CACHE:
        _CACHE["nc"] = _build()
    return _CACHE["nc"]


def _prep_maps(pred, true_u32, centers):
    cb_full = centers[true_u32]  # [B, 2] host-side gather (input-only data)
    in_maps = []
    for c in range(N_CORES):
        lo = c * ROWS_PER_CORE
        hi = lo + ROWS_PER_CORE
        cb_pre = np.ascontiguousarray(
            cb_full[lo:hi].reshape(T, P, 2).transpose(1, 0, 2)
        )
        in_maps.append({
            "pred": pred[lo:hi],
            "cb_pre": cb_pre,
            "centers": centers,
        })
    return in_maps


def kernel(pred, true, centers):
    from concourse.bass_utils import run_bass_kernel_spmd

    pred = np.ascontiguousarray(np.asarray(pred), dtype=np.float32)
    true_u32 = np.asarray(true).astype(np.uint32)
    centers = np.ascontiguousarray(np.asarray(centers), dtype=np.float32)

    in_maps = _prep_maps(pred, true_u32, centers)
    res = run_bass_kernel_spmd(_get_nc(), in_maps, list(range(N_CORES))).results
    total = 0.0
    for r in res:
        total += float(np.sum(r["partial"].astype(np.float64)))
    return np.float32(total)
